# revision 28
# baseline (speedup 1.0000x reference)
"""Day-routed adapter MLP (per-sample day-specific 2-layer MLP + LayerNorm)
for 8 Trainium2 NeuronCores.

Computation per sample b (day d = day_indices[b]):
    h = relu(x[b] @ W1[d] + b1[d])        # [T, D_hid]
    y = h @ W2[d] + b2[d]                 # [T, D_out]
    out = LN(y) * gamma[d] + beta[d]      # LN over last dim

Sharding: data-parallel over batch, 8 samples per core. The per-sample
day weights are gathered on the host (routing is host-visible), and x is
pre-transposed on the host so the device needs no transposes at all:

  pass 1:  hT[h_chunk, :T] += W1[k_chunk, h_chunk].T @ xT[k_chunk, :T]
           (lhsT = W1 natural layout, rhs = xT)  -> hT with H on partitions,
           so b1 is a per-partition bias fused into the ReLU copyback (ACT).
  pass 2:  y[t_tile, :O]  += hT[k_chunk, t_tile].T @ W2[k_chunk, :O]
           (lhsT = hT from pass 1, rhs = W2 natural layout) -> y with T on
           partitions and O on the free axis, which is exactly the layout
           LayerNorm wants (bn_stats/bn_aggr reduce along free axis).

Schedule notes (from TimelineSim traces):
  - Same-day samples are paired per core (host-chosen permutation): a
    pair's W1/W2/b1/b2 tiles load once and both samples reuse them
    (weight DMA drops from 16MB to ~10MB per core; total in+out DMA
    ~28MB -> ~16MB, which also derisks multi-core HBM contention that a
    single-core sim cannot see).
  - Sample-0 x chunks load via SWDGE (Pool-engine desc-gen) in parallel
    with W1 chunks on the serial 625ns/DMA HWDGE, per-chunk interleaved,
    so pass-1 k-chunk matmuls never wait on a monolithic transfer.
  - The PE warms up on an UNINITIALIZED raw SBUF tile (no memset to wait
    on -> busy from ~0.7us): the p-state ramp (full clock only after
    ~3us of continuous PE activity) is paid during the DMA head; any PE
    idle gap would reset the ramp. Warm results go to PSUM banks that
    sample-0's start=True matmuls reset before any read.
  - LayerNorm: middle samples run entirely on DVE (add/stats/TSP) so the
    ACT engine only runs ReLUs in steady state and pass-2 never queues
    behind normalize bursts. The LAST sample's normalize moves to ACT
    (Identity with per-partition scale/bias APs) and its final tile is
    two free-axis PSUM halves with partial bn_stats, so the DVE drains
    before the last matmul stops. rstd uses Abs_reciprocal_sqrt for ALL
    samples — mixing Sqrt and AbsRsqrt tables forces a 1283ns mid-kernel
    LoadActFuncSet (it sits in a different activation table set).
  - y is stored fp16 and upcast on the host: halves output DMA and the
    tail's final transfers (~5e-4 added error vs a 2e-2 budget).
"""

import os

import numpy as np
import ml_dtypes

import concourse.bass as bass
import concourse.mybir as mybir
import concourse.tile as tile
from concourse import bacc
from concourse.bass_utils import run_bass_kernel_spmd

N_CORES = 8
B, T, D_IN = 64, 512, 512
D_HID, D_OUT = 1024, 512
S = B // N_CORES  # samples per core
EPS = 1e-5

P = 128
KD = D_IN // P   # 4 contraction chunks in pass 1
KH = D_HID // P  # 8 contraction chunks in pass 2 (= H chunks of pass 1 out)
MT = T // P      # 4 token tiles in pass 2

# Matmul input dtype. float16: full PE rate (1 cyc/row, FWL hides weight
# loads), half the DMA bytes of fp32, and a 10-bit mantissa (~4x better than
# bf16; fp32 accumulate in PSUM). fp8 was measured at 2.7e-2 rel err (over
# the 2e-2 gate) so DoubleRow is not usable here.
MM_DTYPE = os.environ.get("DAYMLP_MM_DTYPE", "float16")

# Warm-matmul tuning: rhs columns per warm matmul and how many to issue.
WARM_N = int(os.environ.get("DAYMLP_WARM_N", "128"))
WARM_CNT = int(os.environ.get("DAYMLP_WARM_CNT", "28"))

# rstd path: "act" = Abs_reciprocal_sqrt on ACT (no DVE round-trip),
# "dve" = Sqrt on ACT + reciprocal on DVE (baseline path).
RSTD_MODE = os.environ.get("DAYMLP_RSTD", "act")

_cache: dict = {}
last_run_result = None  # stash of BassKernelResults for test harness use


def _build(
    mm_dtype_name: str, apply_affine: bool, rstd_mode: str, shared: tuple
) -> bass.Bass:
    """shared[s] (s>0, s odd) = sample s reuses sample s-1's W1/W2 tiles
    (same-day pair packed by the host) — its W load is skipped entirely."""
    f32 = mybir.dt.float32
    store_dt = getattr(mybir.dt, mm_dtype_name)
    dram_dt = store_dt

    # Bacc (not raw Bass): its compile pipeline moves extra matmul waits onto
    # ldweights and splits >1-wait instructions via event semaphores, which
    # the TRN2 ISA requires.
    nc = bacc.Bacc("TRN2", target_bir_lowering=False)
    # Partition-major DRAM layouts: each SBUF partition's data is one
    # contiguous DRAM run, so every load is 128 large descriptors instead of
    # 128*K small ones (the DMA engines are descriptor-rate limited).
    NW = sum(1 for s in range(S) if not shared[s])  # distinct W loads
    wslot = []
    for s in range(S):
        wslot.append((wslot[-1] if s else -1) if shared[s] else (wslot[-1] + 1 if s else 0))
    xt_d = nc.dram_tensor("xt", [S, P, KD, T], dram_dt, kind="ExternalInput")
    w1_d = nc.dram_tensor("w1", [NW, P, KD, D_HID], dram_dt, kind="ExternalInput")
    b1_d = nc.dram_tensor("b1", [S, P, KH], f32, kind="ExternalInput")
    w2_d = nc.dram_tensor("w2", [NW, P, KH, D_OUT], dram_dt, kind="ExternalInput")
    b2_d = nc.dram_tensor("b2", [S, D_OUT], f32, kind="ExternalInput")
    if apply_affine:
        gm_d = nc.dram_tensor("gm", [S, D_OUT], f32, kind="ExternalInput")
        bt_d = nc.dram_tensor("bt", [S, D_OUT], f32, kind="ExternalInput")
    # fp16 output: the host upcasts to f32. Halves the 8MB/core output
    # DMA and the final tail transfers; adds only ~5e-4 relative error.
    y_d = nc.dram_tensor("y", [S, T, D_OUT], store_dt, kind="ExternalOutput")

    act = mybir.ActivationFunctionType

    with tile.TileContext(nc) as tc:
        with (
            tc.tile_pool(name="xw", bufs=2) as xw,
            tc.tile_pool(name="hb", bufs=2) as hb,
            tc.tile_pool(name="vec", bufs=2) as vec,
            tc.tile_pool(name="yp", bufs=6) as yp,
            tc.tile_pool(name="st", bufs=8) as st,
            tc.tile_pool(name="consts", bufs=1) as cpool,
            tc.tile_pool(name="prologue", bufs=1) as pro,
            tc.tile_pool(name="psum", bufs=8, space="PSUM") as pp,
        ):
            # Sample-0 x/W1 DMAs first in program order: SP issues them to
            # the (serial, 625ns/DMA) HWDGE before anything else, per-chunk
            # and interleaved so chunk k is usable as soon as it lands. The
            # k=0 W1 chunk is split in half so the first 4 matmuls (h<4)
            # only wait for 256KB of weights.

            # Sample-0 x chunks go through SWDGE (Pool-engine desc-gen,
            # ~1us each) while the W1 chunks go through the serial
            # (625ns/DMA) HWDGE — two parallel descriptor paths, so the
            # first matmul's operands are both in flight by ~1us.
            HH = D_HID // 2
            w10a = pro.tile([P, HH], store_dt, tag="w10a", name="w10a")
            nc.sync.dma_start(out=w10a, in_=w1_d[0, :, 0, :HH])
            xt_ck = [pro.tile([P, T], store_dt, tag=f"xt0_{k}", name=f"xt0_{k}")
                     for k in range(KD)]
            nc.gpsimd.dma_start(out=xt_ck[0], in_=xt_d[0, :, 0, :])
            w10b = pro.tile([P, HH], store_dt, tag="w10b", name="w10b")
            nc.sync.dma_start(out=w10b, in_=w1_d[0, :, 0, HH:])
            w1_ck = [None]
            for k in range(1, KD):
                wk = pro.tile([P, D_HID], store_dt, tag=f"w10_{k}", name=f"w10_{k}")
                nc.sync.dma_start(out=wk, in_=w1_d[0, :, k, :])
                nc.gpsimd.dma_start(out=xt_ck[k], in_=xt_d[0, :, k, :])
                w1_ck.append(wk)

            def w1chunk(k, h):
                if k == 0:
                    return (w10a if h < 4 else w10b)[:, P * (h % 4) : P * (h % 4 + 1)]
                return w1_ck[k][:, P * h : P * (h + 1)]

            eps_t = cpool.tile([P, 1], f32)
            nc.gpsimd.memset(eps_t, EPS)

            # PE pre-warm: small matmuls while the first operands are in
            # flight. The p-state ramp needs ~3us of CONTINUOUS PE activity
            # for full clock; any idle gap resets it. The warm tile is a RAW
            # SBUF tensor that is never initialized: the tile framework has
            # no writer to wait on, so the PE starts at ~250ns. The garbage
            # results land in "ps"-tagged PSUM banks that sample 0's
            # start=True matmuls reset before any read.
            with nc.sbuf_tensor([P, max(WARM_N, P)], store_dt) as warm_raw:
                warm_ap = warm_raw[:, :]
                for w in range(WARM_CNT):
                    warm_ps = pp.tile([P, WARM_N], f32, tag="ps", name=f"warm_ps_{w}")
                    nc.tensor.matmul(
                        warm_ps,
                        lhsT=warm_ap[:, :P],
                        rhs=warm_ap[:, :WARM_N],
                        start=True,
                        stop=True,
                    )

            w1c_prev = w2c_prev = None
            for s in range(S):
                if s > 0:
                    xt_t = xw.tile([P, KD, T], store_dt, tag="xt")
                    nc.sync.dma_start(out=xt_t, in_=xt_d[s])
                    if not shared[s]:
                        w1_t = xw.tile([P, KD, D_HID], store_dt, tag="w1")
                        nc.sync.dma_start(out=w1_t, in_=w1_d[wslot[s]])

                def load_rest(s=s, split_w2=False):
                    if split_w2:
                        # s=0 head: half-granularity W2 so pass-2 k<4 matmuls
                        # can start before the k>=4 half lands.
                        w2a = xw.tile([P, KD, D_OUT], store_dt, tag="w2a")
                        nc.sync.dma_start(out=w2a, in_=w2_d[wslot[s], :, :KD, :])
                        b2_t = vec.tile([P, 1, D_OUT], f32, tag="b2")
                        nc.sync.dma_start(
                            out=b2_t, in_=b2_d[s : s + 1, :].partition_broadcast(P)
                        )
                        w2b = xw.tile([P, KD, D_OUT], store_dt, tag="w2b")
                        nc.sync.dma_start(out=w2b, in_=w2_d[wslot[s], :, KD:, :])
                        w2c = lambda k: (w2a if k < KD else w2b)[:, k % KD, :]
                    else:
                        w2_t = xw.tile([P, KH, D_OUT], store_dt, tag="w2")
                        nc.sync.dma_start(out=w2_t, in_=w2_d[wslot[s]])
                        b2_t = vec.tile([P, 1, D_OUT], f32, tag="b2")
                        nc.sync.dma_start(
                            out=b2_t, in_=b2_d[s : s + 1, :].partition_broadcast(P)
                        )
                        w2c = lambda k: w2_t[:, k, :]
                    gm_t = bt_t = None
                    if apply_affine:
                        gm_t = vec.tile([P, 1, D_OUT], f32, tag="gm")
                        nc.sync.dma_start(
                            out=gm_t, in_=gm_d[s : s + 1, :].partition_broadcast(P)
                        )
                        bt_t = vec.tile([P, 1, D_OUT], f32, tag="bt")
                        nc.sync.dma_start(
                            out=bt_t, in_=bt_d[s : s + 1, :].partition_broadcast(P)
                        )
                    return w2c, b2_t, gm_t, bt_t

                if s > 0:
                    # pass-2 operands up front so DMA overlaps pass-1 compute
                    if shared[s]:
                        # same-day pair: W2, b1, b2 (and affine vectors) are
                        # all per-day — reuse every tile from the pair's
                        # first sample. Only x differs.
                        w2c, b2_t, gm_t, bt_t = w2c_prev, b2_prev, gm_prev, bt_prev
                        b1_t = b1_prev
                    else:
                        w2c, b2_t, gm_t, bt_t = load_rest()
                        b1_t = vec.tile([P, KH], f32, tag="b1")
                        nc.sync.dma_start(out=b1_t, in_=b1_d[s])
                else:
                    b1_t = vec.tile([P, KH], f32, tag="b1")

                # pass 1: hT[h, :] = relu(W1[:, h].T @ xT + b1[h])
                hT_t = hb.tile([P, KH, T], store_dt, tag="hT")
                if s == 0:
                    w1c = w1chunk
                    # k-outer over all 8 PSUM banks: matmuls start as soon as
                    # chunk k=0 has landed
                    ps_list = [pp.tile([P, T], f32, tag="ps", name=f"ps0_{h}") for h in range(KH)]
                    for k in range(KD):
                        for h in range(KH):
                            nc.tensor.matmul(
                                ps_list[h],
                                lhsT=w1chunk(k, h),
                                rhs=xt_ck[k],
                                start=(k == 0),
                                stop=(k == KD - 1),
                            )
                    nc.sync.dma_start(out=b1_t, in_=b1_d[s])
                    w2c, b2_t, gm_t, bt_t = load_rest(split_w2=True)
                    for h in range(KH):
                        nc.scalar.activation(
                            out=hT_t[:, h, :],
                            in_=ps_list[h],
                            func=act.Relu,
                            bias=b1_t[:, h : h + 1],
                            scale=1.0,
                        )
                else:
                    if shared[s]:
                        w1c = w1c_prev
                    else:
                        def w1c(k, h, w1_t=w1_t):
                            return w1_t[:, k, P * h : P * (h + 1)]
                    for h in range(KH):
                        ps = pp.tile([P, T], f32, tag="ps")
                        for k in range(KD):
                            nc.tensor.matmul(
                                ps,
                                lhsT=w1c(k, h),
                                rhs=xt_t[:, k, :],
                                start=(k == 0),
                                stop=(k == KD - 1),
                            )
                        nc.scalar.activation(
                            out=hT_t[:, h, :],
                            in_=ps,
                            func=act.Relu,
                            bias=b1_t[:, h : h + 1],
                            scale=1.0,
                        )

                # pass 2: y[t_tile, :] = hT[:, t_tile].T @ W2 (+ b2), then LN.
                # LN work is spread over Pool (+b2), DVE (stats), and ACT
                # (rstd + fused normalize) so no engine serializes the tail.
                # Middle samples keep the whole LN on DVE (baseline style):
                # the ACT engine then only runs ReLUs in steady state, so a
                # sample's pass-2 never queues behind the previous sample's
                # normalize bursts. The LAST sample switches its normalize to
                # ACT (fused Identity with per-partition scale/bias) so the
                # DVE queue can drain before the final matmuls stop, and its
                # final tile is computed as two free-axis PSUM halves whose
                # partial bn_stats start ~850ns before the last matmul.
                last_sample = s == S - 1
                for t in range(MT):
                    last_tile = last_sample and t == MT - 1
                    # Uneven free-axis split on the very last tile: the wide
                    # part's +b2/bn_stats run while its 128-col remainder is
                    # still in the matmul, so only a small add+stats+aggr
                    # chain remains after the final matmul stops.
                    parts = [(0, 384), (384, 128)] if last_tile else [(0, D_OUT)]
                    ps2s = [
                        pp.tile([P, w], f32, tag="ps", name=f"ps2_{s}_{t}_{i}")
                        for i, (off, w) in enumerate(parts)
                    ]
                    y_t = yp.tile([P, D_OUT], store_dt, tag="y")
                    stats = st.tile([P, len(parts), 6], f32, tag="stats")
                    for i, ps2 in enumerate(ps2s):
                        off, w = parts[i]
                        for k in range(KH):
                            nc.tensor.matmul(
                                ps2,
                                lhsT=hT_t[:, k, P * t : P * (t + 1)],
                                rhs=w2c(k)[:, off : off + w],
                                start=(k == 0),
                                stop=(k == KH - 1),
                            )
                        sl = slice(off, off + w)
                        nc.vector.tensor_add(
                            out=y_t[:, sl], in0=ps2, in1=b2_t[:, 0, sl]
                        )
                        nc.vector.bn_stats(out=stats[:, i, :], in_=y_t[:, sl])
                    mv = st.tile([P, 2], f32, tag="mv")
                    nc.vector.bn_aggr(out=mv, in_=stats)
                    rstd = st.tile([P, 1], f32, tag="rstd")
                    # One rstd path for ALL samples: Sqrt and
                    # Abs_reciprocal_sqrt live in different activation table
                    # sets, so mixing them forces a 1283ns mid-kernel
                    # LoadActFuncSet (measured right in the tail).
                    if rstd_mode == "act":
                        nc.scalar.activation(
                            out=rstd,
                            in_=mv[:, 1:2],
                            func=act.Abs_reciprocal_sqrt,
                            bias=eps_t,
                            scale=1.0,
                        )
                    else:
                        nc.scalar.activation(
                            out=rstd,
                            in_=mv[:, 1:2],
                            func=act.Sqrt,
                            bias=eps_t,
                            scale=1.0,
                        )
                        nc.vector.reciprocal(out=rstd, in_=rstd)
                    yo = yp.tile([P, D_OUT], store_dt, tag="yo")
                    if last_sample:
                        # nmr = -mean * rstd (tiny DVE op), then the whole
                        # normalize is one ACT pass: (y*rstd) + nmr.
                        nmr = st.tile([P, 1], f32, tag="nmr")
                        nc.scalar.activation(
                            out=nmr, in_=mv[:, 0:1], func=act.Identity, scale=rstd
                        )
                        nc.scalar.activation(
                            out=nmr, in_=nmr, func=act.Identity, scale=-1.0
                        )
                        nc.scalar.activation(
                            out=yo,
                            in_=y_t,
                            func=act.Identity,
                            bias=nmr,
                            scale=rstd,
                        )
                        if apply_affine:
                            nc.vector.tensor_mul(out=yo, in0=yo, in1=gm_t[:, 0, :])
                            nc.vector.tensor_add(out=yo, in0=yo, in1=bt_t[:, 0, :])
                        nc.sync.dma_start(out=y_d[s, P * t : P * (t + 1), :], in_=yo)
                    else:
                        nc.vector.tensor_scalar(
                            out=yo,
                            in0=y_t,
                            scalar1=mv[:, 0:1],
                            scalar2=rstd,
                            op0=mybir.AluOpType.subtract,
                            op1=mybir.AluOpType.mult,
                        )
                        if apply_affine:
                            nc.vector.tensor_mul(out=yo, in0=yo, in1=gm_t[:, 0, :])
                            nc.vector.tensor_add(out=yo, in0=yo, in1=bt_t[:, 0, :])
                        nc.sync.dma_start(out=y_d[s, P * t : P * (t + 1), :], in_=yo)
                w1c_prev = w1chunk if s == 0 else w1c
                w2c_prev, b2_prev, gm_prev, bt_prev, b1_prev = w2c, b2_t, gm_t, bt_t, b1_t
    nc.finalize()
    return nc


def kernel(**inputs) -> np.ndarray:
    global last_run_result
    x = np.asarray(inputs["x"], dtype=np.float32)
    day = np.asarray(inputs["day_indices"]).astype(np.int64)
    W1 = np.asarray(inputs["W1"], dtype=np.float32)
    b1 = np.asarray(inputs["b1"], dtype=np.float32)
    W2 = np.asarray(inputs["W2"], dtype=np.float32)
    b2 = np.asarray(inputs["b2"], dtype=np.float32)
    gamma = np.asarray(inputs["gamma"], dtype=np.float32)
    beta = np.asarray(inputs["beta"], dtype=np.float32)

    apply_affine = not (np.all(gamma == 1.0) and np.all(beta == 0.0))

    # Same-day pair packing: samples sharing a day are paired on a core so
    # the pair's W1/W2 DMA happens once (skips ~6MB/core of weight traffic).
    # Every core runs the same program, so the shared/indep pattern must be
    # uniform: n_true shared pairs + (4 - n_true) independent pairs per core.
    from collections import defaultdict

    by_day = defaultdict(list)
    for i, d in enumerate(day.tolist()):
        by_day[d].append(i)
    true_pairs = []
    leftover = []
    for d, idxs in sorted(by_day.items()):
        for j in range(0, len(idxs) - 1, 2):
            true_pairs.append((idxs[j], idxs[j + 1]))
        if len(idxs) % 2:
            leftover.append(idxs[-1])
    pairs_per_core = S // 2
    n_true = min(pairs_per_core - 0, len(true_pairs) // N_CORES)
    n_true = min(n_true, pairs_per_core)
    used = true_pairs[: n_true * N_CORES]
    rest = [i for p in true_pairs[n_true * N_CORES :] for i in p] + leftover
    indep_per_core = pairs_per_core - n_true
    # shared-flag pattern per sample position: pair 0 shared first (keeps the
    # head DMA light), indep pairs next, remaining shared pairs last.
    pair_shared = [True] * min(1, n_true) + [False] * indep_per_core + [True] * max(
        0, n_true - 1
    )
    shared = tuple(
        s % 2 == 1 and pair_shared[s // 2] for s in range(S)
    )
    perm = []  # perm[packed_position] = original sample index
    for c in range(N_CORES):
        mine_true = used[c * n_true : (c + 1) * n_true]
        mine_rest = rest[c * 2 * indep_per_core : (c + 1) * 2 * indep_per_core]
        order = []
        ti = 0
        for pr in range(pairs_per_core):
            if pair_shared[pr]:
                order.extend(used[c * n_true + ti])
                ti += 1
            else:
                order.extend(mine_rest[:2])
                mine_rest = mine_rest[2:]
        perm.extend(order)
    perm = np.array(perm, dtype=np.int64)
    day_p = day[perm]

    key = (MM_DTYPE, apply_affine, RSTD_MODE, shared)
    if key not in _cache:
        _cache[key] = _build(*key)
    nc = _cache[key]

    mm_np = {
        "bfloat16": ml_dtypes.bfloat16,
        "float16": np.float16,
    }.get(MM_DTYPE, np.float32)

    # host-side routing gather + layout prep: K on partitions, and
    # partition-major so each partition's DMA data is contiguous in DRAM.
    # All per-sample arrays are in packed (permuted) order.
    xt = np.ascontiguousarray(
        x[perm]
        .transpose(0, 2, 1)
        .reshape(B, KD, P, T)
        .transpose(0, 2, 1, 3)
        .astype(mm_np)
    )
    # W arrays hold one entry per load slot (non-shared sample positions).
    wload = [s for s in range(S) if not shared[s]]
    W1d = np.ascontiguousarray(
        W1.reshape(NUM_DAYS := W1.shape[0], KD, P, D_HID).transpose(0, 2, 1, 3).astype(mm_np)
    )
    W2d = np.ascontiguousarray(
        W2.reshape(NUM_DAYS, KH, P, D_OUT).transpose(0, 2, 1, 3).astype(mm_np)
    )
    b1d = np.ascontiguousarray(b1[day_p].reshape(B, KH, P).transpose(0, 2, 1))
    b2d = np.ascontiguousarray(b2[day_p])
    gmd = np.ascontiguousarray(gamma[day_p])
    btd = np.ascontiguousarray(beta[day_p])

    in_maps = []
    for c in range(N_CORES):
        sl = slice(c * S, (c + 1) * S)
        days_c = day_p[c * S : (c + 1) * S]
        m = {
            "xt": xt[sl],
            "w1": np.ascontiguousarray(W1d[[days_c[s] for s in wload]]),
            "b1": b1d[sl],
            "w2": np.ascontiguousarray(W2d[[days_c[s] for s in wload]]),
            "b2": b2d[sl],
        }
        if apply_affine:
            m["gm"] = gmd[sl]
            m["bt"] = btd[sl]
        in_maps.append(m)

    trace = os.environ.get("DAYMLP_TRACE", "0") == "1"
    res = run_bass_kernel_spmd(
        nc,
        in_maps,
        core_ids=list(range(N_CORES)),
        trace=trace,
    )
    last_run_result = res
    y_packed = np.concatenate([r["y"] for r in res.results], axis=0)
    y = np.empty_like(y_packed)
    y[perm] = y_packed
    return y.astype(np.float32)
